# revision 10
# baseline (speedup 1.0000x reference)
"""Self-contained Trainium2 Bass kernel for the LSS voxel-pooling problem
(nn_DSFusionv2_28819230556604).

kernel(**inputs) takes the FULL unsharded inputs (numpy) and returns the
FULL [B, C, NZ, NY, NX] float32 output.

Strategy (8 NeuronCores, data-parallel over kept (b,n,d,h) rows):
  The camera geometry makes the voxel indices separable per (b,n,d) slice:
  the x,y cell indices depend only on (n,d,w) and the z in-bounds mask only
  on (n,d,h).  The host computes indices (mirroring the reference's float32
  op sequence exactly), drops the rows the reference masks out (~12%), and
  balances the surviving rows across the 8 cores.

  Device pipeline per core (one pass, no DRAM round-trips):
    x rows stream HBM->SBUF in G groups of 128 rows; a 0/1 routing matrix
    Z[g] (lhsT) reduces each group's rows into per-slice-segment column
    sums via PE matmuls accumulating into a single PSUM tile
    [128 segs, 44*80 cols].  Per 512-column chunk the accumulation closes
    after the last group, so the DVE cast (PSUM f32 -> SBUF bf16) and the
    output DMA pipeline right behind the final group's matmuls.

  Host merges the compact per-segment rows (duplicate cells within a slice,
  cross-slice and cross-core duplicates) into the BEV canvas with one
  vectorized scatter-add.
"""
import os
import numpy as np
import ml_dtypes

# ---- problem constants (hardcoded from the reference config) ----
B, N, D, FH, FW, C = 2, 6, 48, 16, 44, 80
OGH, OGW = 256, 704
D_MIN, D_MAX = 2.0, 58.0
NX, NY, NZ = 256, 256, 1
LOWER = np.array([-51.2, -51.2, -10.0], np.float32)
DX = np.array([0.4, 0.4, 20.0], np.float32)

NCORE = 8
WC = FW * C                       # 3520
NSEG = 72                         # per-core output rows (slice segments)
CHUNK = 512                       # psum bank width in f32


def _frustum():
    ds = D_MIN + (D_MAX - D_MIN) / D * np.arange(D, dtype=np.float32)
    ds = np.broadcast_to(ds[:, None, None], (D, FH, FW))
    xs = np.broadcast_to(np.linspace(0, OGW - 1, FW, dtype=np.float32)[None, None, :], (D, FH, FW))
    ys = np.broadcast_to(np.linspace(0, OGH - 1, FH, dtype=np.float32)[None, :, None], (D, FH, FW))
    return np.stack([xs, ys, ds], -1)


def _geometry_indices(rots, trans, intrins, post_rots, post_trans):
    """Voxel indices, bit-matching the reference's float32 op sequence."""
    frustum = _frustum()
    pts = frustum[None, None] - post_trans[:, :, None, None, None, :]
    inv_post = np.linalg.inv(post_rots).astype(np.float32)
    pts = np.einsum('bnij,bndhwj->bndhwi', inv_post, pts).astype(np.float32)
    pts = np.concatenate([pts[..., :2] * pts[..., 2:3], pts[..., 2:3]], axis=-1)
    combine = np.einsum('bnij,bnjk->bnik', rots,
                        np.linalg.inv(intrins).astype(np.float32)).astype(np.float32)
    pts = np.einsum('bnij,bndhwj->bndhwi', combine, pts).astype(np.float32)
    geom = (pts + trans[:, :, None, None, None, :]).astype(np.float32)
    gi = ((geom - LOWER) / DX).astype(np.int32)
    kept = ((gi[..., 0] >= 0) & (gi[..., 0] < NX) &
            (gi[..., 1] >= 0) & (gi[..., 1] < NY) &
            (gi[..., 2] >= 0) & (gi[..., 2] < NZ))
    return gi, kept


def _build_plan(gi, kept):
    """Row-level plan: which (b,n,d,h) x-rows each core reduces, the 0/1
    routing matrices Z, and the per-segment BEV cell indices for the host
    merge."""
    # separability checks (hold for this problem's camera geometry)
    zok = (gi[..., 2] >= 0) & (gi[..., 2] < NZ)             # [B,N,D,FH,FW]
    if not (zok == zok[..., :1]).all():
        raise RuntimeError("structure violation: z-ok varies with w")
    zrow = zok[..., 0]                                       # [B,N,D,FH]
    xyok = ((gi[..., 0] >= 0) & (gi[..., 0] < NX) &
            (gi[..., 1] >= 0) & (gi[..., 1] < NY))
    if not (xyok == xyok[:, :, :, :1, :]).all():
        raise RuntimeError("structure violation: xy-ok varies with h")
    g0 = gi[:, :, :, 0, :, :]                                # [B,N,D,FW,3]
    if not (gi[..., 0] == g0[:, :, :, None, :, 0]).all() or \
       not (gi[..., 1] == g0[:, :, :, None, :, 1]).all():
        raise RuntimeError("structure violation: gi_x/gi_y vary with h")
    if not (kept == (zrow[..., None] & xyok[:, :, :, 0, :][:, :, :, None, :])).all():
        raise RuntimeError("structure violation: kept not separable")

    xyok0 = xyok[:, :, :, 0, :]                              # [B,N,D,FW]
    cellxy = np.where(
        xyok0,
        g0[..., 1].astype(np.int64) * NX + g0[..., 0].astype(np.int64),
        -1)                                                  # [B,N,D,FW]

    # slices with at least one kept row and one valid cell
    nrows = zrow.sum(-1)                                     # [B,N,D]
    hasw = (cellxy >= 0).any(-1)                             # [B,N,D]
    slist = [(int(b), int(n), int(dd), int(nrows[b, n, dd]))
             for b in range(B) for n in range(N) for dd in range(D)
             if nrows[b, n, dd] > 0 and hasw[b, n, dd]]

    # greedy balance rows across cores (largest-first into least-loaded)
    slist.sort(key=lambda t: (-t[3], t[0], t[1], t[2]))
    loads = [0] * NCORE
    assign = [[] for _ in range(NCORE)]
    for b, n, dd, r in slist:
        c = min(range(NCORE), key=lambda i: (loads[i], i))
        assign[c].append((b, n, dd, r))
        loads[c] += r
    G = (max(loads) + 127) // 128

    plans = []
    for core in range(NCORE):
        rowids = np.full(G * 128, -1, np.int64)
        Z = np.zeros((128, G, NSEG), np.float32)
        segcell = np.full((NSEG, FW), -1, np.int64)
        pos = 0
        seg = -1
        for b, n, dd, _r in assign[core]:
            hs = np.nonzero(zrow[b, n, dd])[0]
            newslice = True
            for h in hs:
                g, p = divmod(pos, 128)
                if newslice or p == 0:
                    seg += 1
                    if seg >= NSEG:
                        raise RuntimeError("segment overflow")
                    cells = cellxy[b, n, dd]
                    segcell[seg] = np.where(cells >= 0, cells + b * (NY * NX), -1)
                    newslice = False
                rowids[pos] = ((b * N + n) * D + dd) * FH + int(h)
                Z[p, g, seg] = 1.0
                pos += 1
        plans.append(dict(rowids=rowids, Z=Z, segcell=segcell))
    return plans, G


def _build_nc(G):
    import concourse.bacc as bacc
    import concourse.mybir as mybir
    import concourse.tile as tile
    F32 = mybir.dt.float32
    BF16 = mybir.dt.bfloat16

    nc = bacc.Bacc(None, target_bir_lowering=True)
    x_d = nc.dram_tensor("x", [G * 128, WC], BF16, kind="ExternalInput")
    z_d = nc.dram_tensor("z", [128, G, NSEG], BF16, kind="ExternalInput")
    # chunk-major so each per-chunk output DMA writes one contiguous block
    out_d = nc.dram_tensor("out", [7, NSEG, CHUNK], BF16, kind="ExternalOutput")

    HALF = 2048  # x DMA split (chunk-aligned) so matmuls start early
    with tile.TileContext(nc) as tc:
        with (
            tc.tile_pool(name="sbuf", bufs=1) as pool,
            tc.tile_pool(name="psum", bufs=1, space="PSUM") as psum,
        ):
            ztile = pool.tile([128, G, NSEG], BF16)
            nc.scalar.dma_start(ztile[:], z_d[:])
            xtile = pool.tile([128, G, WC], BF16)
            # per-chunk psum/staging tiles: tile-granular dependency tracking
            # would otherwise serialize every drain behind the last matmul
            pcs = [psum.tile([NSEG, CHUNK], F32, tag=f"ps{c}", name=f"ps{c}")
                   for c in range(7)]
            outs = [pool.tile([NSEG, CHUNK], BF16, name=f"ob{c}")
                    for c in range(7)]

            for g in range(G):
                nc.sync.dma_start(xtile[:, g, 0:HALF],
                                  x_d[128 * g:128 * (g + 1), 0:HALF])
                nc.sync.dma_start(xtile[:, g, HALF:WC],
                                  x_d[128 * g:128 * (g + 1), HALF:WC])
            for g in range(G):
                for c in range(7):
                    o = c * CHUNK
                    w = min(CHUNK, WC - o)
                    nc.tensor.matmul(
                        pcs[c][:, 0:w],
                        ztile[:, g, :], xtile[:, g, o:o + w],
                        start=(g == 0), stop=(g == G - 1),
                        skip_group_check=True,
                    )
            for c in range(7):
                w = min(CHUNK, WC - c * CHUNK)
                if c % 2 == 0:
                    nc.vector.tensor_copy(outs[c][:, 0:w], pcs[c][:, 0:w])
                else:
                    nc.scalar.copy(outs[c][:, 0:w], pcs[c][:, 0:w])
                nc.sync.dma_start(out_d[c][:, 0:w], outs[c][:, 0:w])
    nc.compile()
    return nc


_NC_CACHE = {}
_LAST_EXEC_NS = None


def kernel(x, rots, trans, intrins, post_rots, post_trans):
    global _LAST_EXEC_NS
    x = np.asarray(x)
    rots = np.asarray(rots, np.float32)
    trans = np.asarray(trans, np.float32)
    intrins = np.asarray(intrins, np.float32)
    post_rots = np.asarray(post_rots, np.float32)
    post_trans = np.asarray(post_trans, np.float32)

    gi, kept = _geometry_indices(rots, trans, intrins, post_rots, post_trans)
    plans, G = _build_plan(gi, kept)

    xb = x.astype(ml_dtypes.bfloat16).reshape(B * N * D * FH, WC)
    inmaps = []
    for plan in plans:
        xc = np.zeros((G * 128, WC), ml_dtypes.bfloat16)
        rid = plan["rowids"]
        m = rid >= 0
        xc[m] = xb[rid[m]]
        inmaps.append({
            "x": xc,
            "z": plan["Z"].astype(ml_dtypes.bfloat16),
        })

    if G not in _NC_CACHE:
        _NC_CACHE[G] = _build_nc(G)
    from concourse.bass_utils import run_bass_kernel_spmd
    trace = bool(int(os.environ.get("LSS_TRACE", "0")))
    if not trace:
        # the NTFF trace path needs antenv.axon_hooks, absent in this image;
        # make sure a global BASS_TRACE=1 can't route us there
        os.environ["BASS_NEVER_TRACE"] = "1"
    res = run_bass_kernel_spmd(_NC_CACHE[G], inmaps, core_ids=list(range(NCORE)),
                               trace=trace)
    _LAST_EXEC_NS = res.exec_time_ns

    # host merge: per-segment compact rows -> BEV canvas
    canvas = np.zeros((B * NY * NX, C), np.float64)
    for r, plan in zip(res.results, plans):
        och = np.asarray(r["out"])               # [7, NSEG, CHUNK] bf16
        flat = np.concatenate(
            [och[c][:, :min(CHUNK, WC - c * CHUNK)] for c in range(7)], axis=1)
        vals = flat.astype(np.float32).reshape(NSEG, FW, C)
        idx = plan["segcell"]                    # [NSEG, FW]
        m = idx >= 0
        np.add.at(canvas, idx[m], vals[m].astype(np.float64))
    out = (canvas.reshape(B, NY, NX, C).transpose(0, 3, 1, 2)[:, :, None]
           .astype(np.float32))
    return np.ascontiguousarray(out.reshape(B, C, NZ, NY, NX))


# revision 12
# speedup vs baseline: 1.0008x; 1.0008x over previous
"""Self-contained Trainium2 Bass kernel for the LSS voxel-pooling problem
(nn_DSFusionv2_28819230556604).

kernel(**inputs) takes the FULL unsharded inputs (numpy) and returns the
FULL [B, C, NZ, NY, NX] float32 output.

Strategy (8 NeuronCores, data-parallel over kept (b,n,d,h) rows):
  The camera geometry makes the voxel indices separable per (b,n,d) slice:
  the x,y cell indices depend only on (n,d,w) and the z in-bounds mask only
  on (n,d,h).  The host computes indices (mirroring the reference's float32
  op sequence exactly), drops the rows the reference masks out (~12%), and
  balances the surviving rows across the 8 cores.

  Device pipeline per core (one pass, no DRAM round-trips):
    x rows stream HBM->SBUF in G groups of 128 rows; a 0/1 routing matrix
    Z[g] (lhsT) reduces each group's rows into per-slice-segment column
    sums via PE matmuls accumulating into a single PSUM tile
    [128 segs, 44*80 cols].  Per 512-column chunk the accumulation closes
    after the last group, so the DVE cast (PSUM f32 -> SBUF bf16) and the
    output DMA pipeline right behind the final group's matmuls.

  Host merges the compact per-segment rows (duplicate cells within a slice,
  cross-slice and cross-core duplicates) into the BEV canvas with one
  vectorized scatter-add.
"""
import os
import numpy as np
import ml_dtypes

# ---- problem constants (hardcoded from the reference config) ----
B, N, D, FH, FW, C = 2, 6, 48, 16, 44, 80
OGH, OGW = 256, 704
D_MIN, D_MAX = 2.0, 58.0
NX, NY, NZ = 256, 256, 1
LOWER = np.array([-51.2, -51.2, -10.0], np.float32)
DX = np.array([0.4, 0.4, 20.0], np.float32)

NCORE = 8
WC = FW * C                       # 3520
NSEG = 72                         # per-core output rows (slice segments)
CHUNK = 512                       # psum bank width in f32


def _frustum():
    ds = D_MIN + (D_MAX - D_MIN) / D * np.arange(D, dtype=np.float32)
    ds = np.broadcast_to(ds[:, None, None], (D, FH, FW))
    xs = np.broadcast_to(np.linspace(0, OGW - 1, FW, dtype=np.float32)[None, None, :], (D, FH, FW))
    ys = np.broadcast_to(np.linspace(0, OGH - 1, FH, dtype=np.float32)[None, :, None], (D, FH, FW))
    return np.stack([xs, ys, ds], -1)


def _geometry_indices(rots, trans, intrins, post_rots, post_trans):
    """Voxel indices, bit-matching the reference's float32 op sequence."""
    frustum = _frustum()
    pts = frustum[None, None] - post_trans[:, :, None, None, None, :]
    inv_post = np.linalg.inv(post_rots).astype(np.float32)
    pts = np.einsum('bnij,bndhwj->bndhwi', inv_post, pts).astype(np.float32)
    pts = np.concatenate([pts[..., :2] * pts[..., 2:3], pts[..., 2:3]], axis=-1)
    combine = np.einsum('bnij,bnjk->bnik', rots,
                        np.linalg.inv(intrins).astype(np.float32)).astype(np.float32)
    pts = np.einsum('bnij,bndhwj->bndhwi', combine, pts).astype(np.float32)
    geom = (pts + trans[:, :, None, None, None, :]).astype(np.float32)
    gi = ((geom - LOWER) / DX).astype(np.int32)
    kept = ((gi[..., 0] >= 0) & (gi[..., 0] < NX) &
            (gi[..., 1] >= 0) & (gi[..., 1] < NY) &
            (gi[..., 2] >= 0) & (gi[..., 2] < NZ))
    return gi, kept


def _build_plan(gi, kept):
    """Row-level plan: which (b,n,d,h) x-rows each core reduces, the 0/1
    routing matrices Z, and the per-segment BEV cell indices for the host
    merge."""
    # separability checks (hold for this problem's camera geometry)
    zok = (gi[..., 2] >= 0) & (gi[..., 2] < NZ)             # [B,N,D,FH,FW]
    if not (zok == zok[..., :1]).all():
        raise RuntimeError("structure violation: z-ok varies with w")
    zrow = zok[..., 0]                                       # [B,N,D,FH]
    xyok = ((gi[..., 0] >= 0) & (gi[..., 0] < NX) &
            (gi[..., 1] >= 0) & (gi[..., 1] < NY))
    if not (xyok == xyok[:, :, :, :1, :]).all():
        raise RuntimeError("structure violation: xy-ok varies with h")
    g0 = gi[:, :, :, 0, :, :]                                # [B,N,D,FW,3]
    if not (gi[..., 0] == g0[:, :, :, None, :, 0]).all() or \
       not (gi[..., 1] == g0[:, :, :, None, :, 1]).all():
        raise RuntimeError("structure violation: gi_x/gi_y vary with h")
    if not (kept == (zrow[..., None] & xyok[:, :, :, 0, :][:, :, :, None, :])).all():
        raise RuntimeError("structure violation: kept not separable")

    xyok0 = xyok[:, :, :, 0, :]                              # [B,N,D,FW]
    cellxy = np.where(
        xyok0,
        g0[..., 1].astype(np.int64) * NX + g0[..., 0].astype(np.int64),
        -1)                                                  # [B,N,D,FW]

    # slices with at least one kept row and one valid cell
    nrows = zrow.sum(-1)                                     # [B,N,D]
    hasw = (cellxy >= 0).any(-1)                             # [B,N,D]
    slist = [(int(b), int(n), int(dd), int(nrows[b, n, dd]))
             for b in range(B) for n in range(N) for dd in range(D)
             if nrows[b, n, dd] > 0 and hasw[b, n, dd]]

    # greedy balance rows across cores (largest-first into least-loaded)
    slist.sort(key=lambda t: (-t[3], t[0], t[1], t[2]))
    loads = [0] * NCORE
    assign = [[] for _ in range(NCORE)]
    for b, n, dd, r in slist:
        c = min(range(NCORE), key=lambda i: (loads[i], i))
        assign[c].append((b, n, dd, r))
        loads[c] += r
    G = (max(loads) + 127) // 128

    plans = []
    for core in range(NCORE):
        rowids = np.full(G * 128, -1, np.int64)
        Z = np.zeros((128, G, NSEG), np.float32)
        segcell = np.full((NSEG, FW), -1, np.int64)
        pos = 0
        seg = -1
        for b, n, dd, _r in assign[core]:
            hs = np.nonzero(zrow[b, n, dd])[0]
            newslice = True
            for h in hs:
                g, p = divmod(pos, 128)
                if newslice or p == 0:
                    seg += 1
                    if seg >= NSEG:
                        raise RuntimeError("segment overflow")
                    cells = cellxy[b, n, dd]
                    segcell[seg] = np.where(cells >= 0, cells + b * (NY * NX), -1)
                    newslice = False
                rowids[pos] = ((b * N + n) * D + dd) * FH + int(h)
                Z[p, g, seg] = 1.0
                pos += 1
        plans.append(dict(rowids=rowids, Z=Z, segcell=segcell))
    return plans, G


def _build_nc(G):
    import concourse.bacc as bacc
    import concourse.mybir as mybir
    import concourse.tile as tile
    F32 = mybir.dt.float32
    BF16 = mybir.dt.bfloat16

    nc = bacc.Bacc(None, target_bir_lowering=True)
    x_d = nc.dram_tensor("x", [G * 128, WC], BF16, kind="ExternalInput")
    z_d = nc.dram_tensor("z", [128, G, NSEG], BF16, kind="ExternalInput")
    # seg-major, chunk-padded layout so the two output DMAs are contiguous
    out_d = nc.dram_tensor("out", [NSEG, 7, CHUNK], BF16, kind="ExternalOutput")

    with tile.TileContext(nc) as tc:
        with (
            tc.tile_pool(name="sbuf", bufs=1) as pool,
            tc.tile_pool(name="psum", bufs=1, space="PSUM") as psum,
        ):
            ztile = pool.tile([128, G, NSEG], BF16)
            nc.scalar.dma_start(ztile[:], z_d[:])
            xtile = pool.tile([128, G, WC], BF16)
            # per-chunk psum tiles: tile-granular dependency tracking would
            # otherwise serialize every drain behind the last matmul
            pcs = [psum.tile([NSEG, CHUNK], F32, tag=f"ps{c}", name=f"ps{c}")
                   for c in range(7)]
            # two staging tiles -> two wide output DMAs (vector drains 0-3,
            # scalar drains 4-6, DMAs on separate HWDGE queues)
            obA = pool.tile([NSEG, 4, CHUNK], BF16)
            obB = pool.tile([NSEG, 3, CHUNK], BF16)

            for g in range(G):
                nc.sync.dma_start(xtile[:, g, :], x_d[128 * g:128 * (g + 1), :])
            for g in range(G):
                for c in range(7):
                    o = c * CHUNK
                    w = min(CHUNK, WC - o)
                    nc.tensor.matmul(
                        pcs[c][:, 0:w],
                        ztile[:, g, :], xtile[:, g, o:o + w],
                        start=(g == 0), stop=(g == G - 1),
                        skip_group_check=True,
                    )
            for c in range(4):
                nc.vector.tensor_copy(obA[:, c, :], pcs[c][:])
            for c in range(4, 7):
                w = min(CHUNK, WC - c * CHUNK)
                nc.scalar.copy(obB[:, c - 4, 0:w], pcs[c][:, 0:w])
            nc.sync.dma_start(out_d[:, 0:4, :], obA[:])
            nc.scalar.dma_start(out_d[:, 4:7, :], obB[:])
    nc.compile()
    return nc


_NC_CACHE = {}
_LAST_EXEC_NS = None


def kernel(x, rots, trans, intrins, post_rots, post_trans):
    global _LAST_EXEC_NS
    x = np.asarray(x)
    rots = np.asarray(rots, np.float32)
    trans = np.asarray(trans, np.float32)
    intrins = np.asarray(intrins, np.float32)
    post_rots = np.asarray(post_rots, np.float32)
    post_trans = np.asarray(post_trans, np.float32)

    gi, kept = _geometry_indices(rots, trans, intrins, post_rots, post_trans)
    plans, G = _build_plan(gi, kept)

    xb = x.astype(ml_dtypes.bfloat16).reshape(B * N * D * FH, WC)
    inmaps = []
    for plan in plans:
        xc = np.zeros((G * 128, WC), ml_dtypes.bfloat16)
        rid = plan["rowids"]
        m = rid >= 0
        xc[m] = xb[rid[m]]
        inmaps.append({
            "x": xc,
            "z": plan["Z"].astype(ml_dtypes.bfloat16),
        })

    if G not in _NC_CACHE:
        _NC_CACHE[G] = _build_nc(G)
    from concourse.bass_utils import run_bass_kernel_spmd
    trace = bool(int(os.environ.get("LSS_TRACE", "0")))
    if not trace:
        # the NTFF trace path needs antenv.axon_hooks, absent in this image;
        # make sure a global BASS_TRACE=1 can't route us there
        os.environ["BASS_NEVER_TRACE"] = "1"
    res = run_bass_kernel_spmd(_NC_CACHE[G], inmaps, core_ids=list(range(NCORE)),
                               trace=trace)
    _LAST_EXEC_NS = res.exec_time_ns

    # host merge: per-segment compact rows -> BEV canvas
    canvas = np.zeros((B * NY * NX, C), np.float64)
    for r, plan in zip(res.results, plans):
        och = np.asarray(r["out"])               # [NSEG, 7, CHUNK] bf16
        flat = np.concatenate(
            [och[:, c, :min(CHUNK, WC - c * CHUNK)] for c in range(7)], axis=1)
        vals = flat.astype(np.float32).reshape(NSEG, FW, C)
        idx = plan["segcell"]                    # [NSEG, FW]
        m = idx >= 0
        np.add.at(canvas, idx[m], vals[m].astype(np.float64))
    out = (canvas.reshape(B, NY, NX, C).transpose(0, 3, 1, 2)[:, :, None]
           .astype(np.float32))
    return np.ascontiguousarray(out.reshape(B, C, NZ, NY, NX))
